# revision 2
# baseline (speedup 1.0000x reference)
"""Trainium2 Bass kernel for nn_LocallyConnectedBlock.

Locally-connected conv (5x5, stride 2, SAME) + bias + leaky_relu(0.01) +
BatchNorm (training mode, batch stats over B,OH,OW).

Reference shapes:
  x      [B=32, H=64, W=64, C=32]
  kernel [OH=32, OW=32, 800, F=64]   (fan_in c-major: k = c*25 + kh*5 + kw)
  bias   [OH=32, OW=32, F=64]
  scale  [F], bn_bias [F]
  out    [B=32, OH=32, OW=32, F=64]

Sharding: spatial over OH, 4 output rows per core x 8 cores. Each core:
  - 128 output positions (ohl in 0..3, ow in 0..31)
  - per position: out[f, b] = sum_k kernel_chunk[k, f].T @ patches_chunk[k, b]
    7 chunks: 6x128 (kh 0..3 x kw 0..4 via shifted x replicas, + kh=4 x kw 0..3)
    + 1x64 tail (tap (4,4), bias row via ones, zero pad)
  - leaky relu + BN partial sums on device, [64,2] AllReduce across cores,
    normalize on device. Host only marshals layouts.
"""

import numpy as np

import concourse.bass as bass
import concourse.mybir as mybir
import concourse.tile as tile
from concourse import bacc
from concourse import bass_utils

B, H, W, CIN = 32, 64, 64, 32
KH = KW = 5
F = 64
OH = OW = 32
NCORES = 8
OHL = 4  # output rows per core
NPOS = OHL * OW  # 128 positions per core
NEG_SLOPE = 0.01
EPS = 1e-5
NTOT = float(B * OH * OW)  # BN sample count (32768)
GROUPS = 16
GP = NPOS // GROUPS  # 8 positions per group

F32 = mybir.dt.float32


def _marshal(x, kern, bias):
    """Build the 8 per-core input maps (all float32, C-contiguous)."""
    x = np.ascontiguousarray(x, dtype=np.float32)
    kern = np.ascontiguousarray(kern, dtype=np.float32)
    bias = np.ascontiguousarray(bias, dtype=np.float32)

    # SAME padding for 5x5 stride2: pad_lo=1, pad_hi=2 (verified vs jax)
    xp = np.zeros((B, H + 3, W + 3, CIN), np.float32)
    xp[:, 1 : 1 + H, 1 : 1 + W, :] = x
    # patch(oh,ow,kh,kw,c) = xp[:, 2*oh+kh, 2*ow+kw, c]

    kr = kern.reshape(OH, OW, CIN, KH, KW, F)  # c-major fan_in (verified)

    jj = np.arange(4)
    in_maps = []
    for c in range(NCORES):
        r0 = 8 * c
        # XH[j*32+ci, ohl, w, b] = xp[b, r0+2*ohl+j, w, ci]   (w in 0..66)
        rows = r0 + 2 * jj[None, :] + jj[:, None]  # [j, ohl]
        t = xp[:, rows, 0:67, :]  # [B, j, ohl, 67, CIN]
        xh = np.ascontiguousarray(t.transpose(1, 4, 2, 3, 0)).reshape(128, -1)

        # rows for kh=4 taps
        rw = r0 + 2 * jj + 4  # [ohl]
        t2 = xp[:, rw, :, :]  # [B, ohl, W+3, CIN]
        # XW[j*32+ci, ohl, ow, b] = xp[b, r0+2*ohl+4, 2*ow+j, ci]
        colidx = 2 * np.arange(OW)[None, :] + jj[:, None]  # [j, ow]
        t3 = t2[:, :, colidx, :]  # [B, ohl, j, ow, CIN]
        xw = np.ascontiguousarray(t3.transpose(2, 4, 1, 3, 0)).reshape(128, -1)

        # XR[ci, ohl, ow, b] = xp[b, r0+2*ohl+4, 2*ow+4, ci]; row32=1; 33..63=0
        t4 = t2[:, :, 2 * np.arange(OW) + 4, :]  # [B, ohl, ow, CIN]
        xr = np.zeros((64, OHL, OW, B), np.float32)
        xr[0:32] = t4.transpose(3, 1, 2, 0)
        xr[32] = 1.0
        xr = xr.reshape(64, -1)

        ks = kr[4 * c : 4 * c + 4]  # [ohl, ow, ci, kh, kw, f]
        # KM[j*32+ci, pos, t, f]: t<5 -> (kh=j, kw=t); t=5 -> (kh=4, kw=j)
        km = np.empty((4, 32, OHL, OW, 6, F), np.float32)  # [j, ci, ohl, ow, t, f]
        for tt in range(5):
            km[:, :, :, :, tt, :] = ks[:, :, :, 0:4, tt, :].transpose(3, 2, 0, 1, 4)
        km[:, :, :, :, 5, :] = ks[:, :, :, 4, 0:4, :].transpose(3, 2, 0, 1, 4)
        km = np.ascontiguousarray(km).reshape(128, NPOS, 6, F).reshape(128, -1)

        # KT[p, pos, f]: p<32 tap(4,4); p=32 bias; rest 0
        kt = np.zeros((64, OHL, OW, F), np.float32)
        kt[0:32] = ks[:, :, :, 4, 4, :].transpose(2, 0, 1, 3)
        kt[32] = bias[4 * c : 4 * c + 4]
        kt = kt.reshape(64, -1)

        in_maps.append({"XH": xh, "XW": xw, "XR": xr, "KM": km, "KT": kt})
    return in_maps


def _build_nc():
    nc = bacc.Bacc(
        "TRN2",
        target_bir_lowering=False,
        debug=False,
        enable_asserts=False,
        num_devices=NCORES,
    )
    XH = nc.dram_tensor("XH", [128, OHL * 67 * CIN], F32, kind="ExternalInput")
    XW = nc.dram_tensor("XW", [128, OHL * OW * B], F32, kind="ExternalInput")
    XR = nc.dram_tensor("XR", [64, OHL * OW * B], F32, kind="ExternalInput")
    KM = nc.dram_tensor("KM", [128, NPOS * 6 * F], F32, kind="ExternalInput")
    KT = nc.dram_tensor("KT", [64, NPOS * F], F32, kind="ExternalInput")
    SC = nc.dram_tensor("SC", [64, 1], F32, kind="ExternalInput")
    BB = nc.dram_tensor("BB", [64, 1], F32, kind="ExternalInput")
    Y = nc.dram_tensor("Y", [64, NPOS * B], F32, kind="ExternalOutput")

    mult = mybir.AluOpType.mult
    amax = mybir.AluOpType.max
    aadd = mybir.AluOpType.add

    with tile.TileContext(nc) as tc:
        with (
            tc.tile_pool(name="singles", bufs=1) as singles,
            tc.tile_pool(name="kmp", bufs=3) as kmp,
            tc.tile_pool(name="scratch", bufs=2) as scratch,
            tc.tile_pool(name="small", bufs=1) as small,
            tc.tile_pool(name="psum", bufs=4, space=bass.MemorySpace.PSUM) as psp,
            tc.tile_pool(name="dram", bufs=1, space=bass.MemorySpace.DRAM) as dram,
        ):
            xh = singles.tile([128, OHL, 67, CIN], F32)
            nc.sync.dma_start(out=xh[:], in_=XH.ap().rearrange("p (a b c) -> p a b c", a=OHL, b=67))
            xw = singles.tile([128, OHL, OW, B], F32)
            nc.sync.dma_start(out=xw[:], in_=XW.ap().rearrange("p (a b c) -> p a b c", a=OHL, b=OW))
            xr = singles.tile([64, OHL, OW, B], F32)
            nc.sync.dma_start(out=xr[:], in_=XR.ap().rearrange("p (a b c) -> p a b c", a=OHL, b=OW))
            kt = singles.tile([64, NPOS, F], F32)
            nc.sync.dma_start(out=kt[:], in_=KT.ap().rearrange("p (a b) -> p a b", a=NPOS))
            sc = small.tile([64, 1], F32)
            nc.sync.dma_start(out=sc[:], in_=SC.ap())
            bb = small.tile([64, 1], F32)
            nc.sync.dma_start(out=bb[:], in_=BB.ap())

            y_sb = singles.tile([64, NPOS, B], F32)
            sums = small.tile([64, GROUPS], F32)
            sqsums = small.tile([64, GROUPS], F32)

            kmv = KM.ap().rearrange(
                "p (g q t f) -> p g q t f", g=GROUPS, q=GP, t=6
            )

            for g in range(GROUPS):
                km = kmp.tile([128, GP, 6, F], F32)
                nc.sync.dma_start(out=km[:], in_=kmv[:, g])
                ps = psp.tile([64, GP, B], F32)
                for pl in range(GP):
                    pos = g * GP + pl
                    ohl, ow = divmod(pos, OW)
                    for t in range(6):
                        rhs = (
                            xh[:, ohl, 2 * ow + t, :]
                            if t < 5
                            else xw[:, ohl, ow, :]
                        )
                        nc.tensor.matmul(
                            ps[:, pl, :],
                            km[:, pl, t, :],
                            rhs,
                            start=(t == 0),
                            stop=False,
                        )
                    nc.tensor.matmul(
                        ps[:, pl, :],
                        kt[:, pos, :],
                        xr[:, ohl, ow, :],
                        start=False,
                        stop=True,
                    )
                # leaky relu drain: y = max(ps, 0.01*ps), accumulate sums
                tmp = scratch.tile([64, GP, B], F32, tag="lr")
                nc.scalar.activation(
                    out=tmp[:], in_=ps[:], func=mybir.ActivationFunctionType.Copy,
                    scale=NEG_SLOPE,
                )
                ysl = y_sb[:, g * GP : (g + 1) * GP, :]
                nc.vector.scalar_tensor_tensor(
                    out=ysl,
                    in0=ps[:],
                    scalar=1.0,
                    in1=tmp[:],
                    op0=mult,
                    op1=amax,
                    accum_out=sums[:, g : g + 1],
                )
                sq = scratch.tile([64, GP, B], F32, tag="sq")
                nc.scalar.activation(
                    out=sq[:], in_=ysl, func=mybir.ActivationFunctionType.Square,
                    accum_out=sqsums[:, g : g + 1],
                )

            # ---- BN stats: local totals -> AllReduce -> normalize ----
            cc_sb = small.tile([64, 2], F32)
            nc.vector.tensor_reduce(
                out=cc_sb[:, 0:1], in_=sums[:], axis=mybir.AxisListType.X, op=aadd
            )
            nc.vector.tensor_reduce(
                out=cc_sb[:, 1:2], in_=sqsums[:], axis=mybir.AxisListType.X, op=aadd
            )
            ccin = dram.tile([64, 2], F32)
            ccout = dram.tile([64, 2], F32)
            nc.sync.dma_start(out=ccin[:], in_=cc_sb[:])
            nc.gpsimd.collective_compute(
                "AllReduce",
                aadd,
                replica_groups=[list(range(NCORES))],
                ins=[ccin.opt()],
                outs=[ccout.opt()],
            )
            tot = small.tile([64, 2], F32)
            nc.sync.dma_start(out=tot[:], in_=ccout[:])

            # mean/meansq
            ms = small.tile([64, 2], F32)
            nc.scalar.activation(
                out=ms[:], in_=tot[:], func=mybir.ActivationFunctionType.Copy,
                scale=1.0 / NTOT,
            )
            var = small.tile([64, 1], F32)
            nc.vector.tensor_mul(var[:], ms[:, 0:1], ms[:, 0:1])
            nc.vector.tensor_sub(var[:], ms[:, 1:2], var[:])
            epst = small.tile([64, 1], F32)
            nc.vector.memset(epst[:], EPS)
            sd = small.tile([64, 1], F32)
            nc.scalar.activation(
                out=sd[:], in_=var[:], func=mybir.ActivationFunctionType.Sqrt,
                bias=epst[:], scale=1.0,
            )
            rstd = small.tile([64, 1], F32)
            nc.vector.reciprocal(out=rstd[:], in_=sd[:])
            av = small.tile([64, 1], F32)
            nc.vector.tensor_mul(av[:], sc[:], rstd[:])
            bv = small.tile([64, 1], F32)
            nc.vector.tensor_mul(bv[:], ms[:, 0:1], av[:])
            nc.vector.tensor_sub(bv[:], bb[:], bv[:])

            yo = singles.tile([64, NPOS, B], F32)
            nc.scalar.activation(
                out=yo[:], in_=y_sb[:],
                func=mybir.ActivationFunctionType.Identity,
                bias=bv[:], scale=av[:],
            )
            nc.sync.dma_start(
                out=Y.ap().rearrange("p (a b) -> p a b", a=NPOS), in_=yo[:]
            )

    nc.compile()
    return nc


_NC_CACHE = None
RUN_KWARGS = {}  # test harness may set e.g. {"trace": True}
LAST_RESULT = None


def kernel(x, kernel, bias, scale, bn_bias):
    global _NC_CACHE, LAST_RESULT
    in_maps = _marshal(x, kernel, bias)
    sc = np.ascontiguousarray(np.asarray(scale, np.float32).reshape(64, 1))
    bb = np.ascontiguousarray(np.asarray(bn_bias, np.float32).reshape(64, 1))
    for m in in_maps:
        m["SC"] = sc
        m["BB"] = bb

    if _NC_CACHE is None:
        _NC_CACHE = _build_nc()
    nc = _NC_CACHE

    res = bass_utils.run_bass_kernel_spmd(
        nc, in_maps, core_ids=list(range(NCORES)), **RUN_KWARGS
    )
    LAST_RESULT = res

    out = np.empty((B, OH, OW, F), np.float32)
    for c in range(NCORES):
        yc = res.results[c]["Y"].reshape(64, OHL, OW, B)  # [f, ohl, ow, b]
        out[:, 4 * c : 4 * c + 4, :, :] = yc.transpose(3, 1, 2, 0)
    return out


# revision 4
# speedup vs baseline: 1.7666x; 1.7666x over previous
"""Trainium2 Bass kernel for nn_LocallyConnectedBlock.

Locally-connected conv (5x5, stride 2, SAME) + bias + leaky_relu(0.01) +
BatchNorm (training mode, batch stats over B,OH,OW).

Reference shapes:
  x      [B=32, H=64, W=64, C=32]
  kernel [OH=32, OW=32, 800, F=64]   (fan_in c-major: k = c*25 + kh*5 + kw)
  bias   [OH=32, OW=32, F=64]
  scale  [F], bn_bias [F]
  out    [B=32, OH=32, OW=32, F=64]

Sharding: spatial over OH, 4 output rows per core x 8 cores. Each core:
  - 128 output positions (ohl in 0..3, ow in 0..31)
  - per position: out[f, b] = sum_k kernel_chunk[k, f].T @ patches_chunk[k, b]
    7 chunks: 6x128 (kh 0..3 x kw 0..4 via shifted x replicas, + kh=4 x kw 0..3)
    + 1x64 tail (tap (4,4), bias row via ones, zero pad)
  - leaky relu + BN partial sums on device, [64,2] AllReduce across cores,
    normalize on device. Host only marshals layouts.
"""

import ml_dtypes
import numpy as np

import concourse.bass as bass
import concourse.mybir as mybir
import concourse.tile as tile
from concourse import bacc
from concourse import bass_utils

B, H, W, CIN = 32, 64, 64, 32
KH = KW = 5
F = 64
OH = OW = 32
NCORES = 8
OHL = 4  # output rows per core
NPOS = OHL * OW  # 128 positions per core
NEG_SLOPE = 0.01
EPS = 1e-5
NTOT = float(B * OH * OW)  # BN sample count (32768)
GROUPS = 8
GP = NPOS // GROUPS  # 16 positions per group

F32 = mybir.dt.float32
BF16 = mybir.dt.bfloat16


def _marshal(x, kern, bias):
    """Build the 8 per-core input maps (all float32, C-contiguous)."""
    x = np.ascontiguousarray(x, dtype=np.float32)
    kern = np.ascontiguousarray(kern, dtype=np.float32)
    bias = np.ascontiguousarray(bias, dtype=np.float32)

    # SAME padding for 5x5 stride2: pad_lo=1, pad_hi=2 (verified vs jax)
    xp = np.zeros((B, H + 3, W + 3, CIN), np.float32)
    xp[:, 1 : 1 + H, 1 : 1 + W, :] = x
    # patch(oh,ow,kh,kw,c) = xp[:, 2*oh+kh, 2*ow+kw, c]

    kr = kern.reshape(OH, OW, CIN, KH, KW, F)  # c-major fan_in (verified)

    jj = np.arange(4)
    in_maps = []
    for c in range(NCORES):
        r0 = 8 * c
        # XH[j*32+ci, ohl, w, b] = xp[b, r0+2*ohl+j, w, ci]   (w in 0..66)
        rows = r0 + 2 * jj[None, :] + jj[:, None]  # [j, ohl]
        t = xp[:, rows, 0:67, :]  # [B, j, ohl, 67, CIN]
        xh = np.ascontiguousarray(t.transpose(1, 4, 2, 3, 0)).reshape(128, -1)

        # rows for kh=4 taps
        rw = r0 + 2 * jj + 4  # [ohl]
        t2 = xp[:, rw, :, :]  # [B, ohl, W+3, CIN]
        # XW[j*32+ci, ohl, ow, b] = xp[b, r0+2*ohl+4, 2*ow+j, ci]
        colidx = 2 * np.arange(OW)[None, :] + jj[:, None]  # [j, ow]
        t3 = t2[:, :, colidx, :]  # [B, ohl, j, ow, CIN]
        xw = np.ascontiguousarray(t3.transpose(2, 4, 1, 3, 0)).reshape(128, -1)

        # XR[ci, ohl, ow, b] = xp[b, r0+2*ohl+4, 2*ow+4, ci]; row32=1; 33..63=0
        t4 = t2[:, :, 2 * np.arange(OW) + 4, :]  # [B, ohl, ow, CIN]
        xr = np.zeros((64, OHL, OW, B), np.float32)
        xr[0:32] = t4.transpose(3, 1, 2, 0)
        xr[32] = 1.0
        xr = xr.reshape(64, -1)

        ks = kr[4 * c : 4 * c + 4]  # [ohl, ow, ci, kh, kw, f]
        # KM[j*32+ci, pos, t, f]: t<5 -> (kh=j, kw=t); t=5 -> (kh=4, kw=j)
        km = np.empty((4, 32, OHL, OW, 6, F), np.float32)  # [j, ci, ohl, ow, t, f]
        for tt in range(5):
            km[:, :, :, :, tt, :] = ks[:, :, :, 0:4, tt, :].transpose(3, 2, 0, 1, 4)
        km[:, :, :, :, 5, :] = ks[:, :, :, 4, 0:4, :].transpose(3, 2, 0, 1, 4)
        km = np.ascontiguousarray(km).reshape(128, NPOS, 6, F).reshape(128, -1)

        # KT[p, pos, f]: p<32 tap(4,4); p=32 bias; rest 0
        kt = np.zeros((64, OHL, OW, F), np.float32)
        kt[0:32] = ks[:, :, :, 4, 4, :].transpose(2, 0, 1, 3)
        kt[32] = bias[4 * c : 4 * c + 4]
        kt = kt.reshape(64, -1)

        bf = lambda a: np.ascontiguousarray(a.astype(ml_dtypes.bfloat16))
        in_maps.append({"XH": bf(xh), "XW": bf(xw), "XR": bf(xr), "KM": bf(km), "KT": bf(kt)})
    return in_maps


def _build_nc():
    nc = bacc.Bacc(
        "TRN2",
        target_bir_lowering=False,
        debug=False,
        enable_asserts=False,
        num_devices=NCORES,
    )
    XH = nc.dram_tensor("XH", [128, OHL * 67 * CIN], BF16, kind="ExternalInput")
    XW = nc.dram_tensor("XW", [128, OHL * OW * B], BF16, kind="ExternalInput")
    XR = nc.dram_tensor("XR", [64, OHL * OW * B], BF16, kind="ExternalInput")
    KM = nc.dram_tensor("KM", [128, NPOS * 6 * F], BF16, kind="ExternalInput")
    KT = nc.dram_tensor("KT", [64, NPOS * F], BF16, kind="ExternalInput")
    SC = nc.dram_tensor("SC", [64, 1], F32, kind="ExternalInput")
    BB = nc.dram_tensor("BB", [64, 1], F32, kind="ExternalInput")
    Y = nc.dram_tensor("Y", [64, NPOS * B], F32, kind="ExternalOutput")

    mult = mybir.AluOpType.mult
    amax = mybir.AluOpType.max
    aadd = mybir.AluOpType.add

    with tile.TileContext(nc) as tc:
        with (
            tc.tile_pool(name="singles", bufs=1) as singles,
            tc.tile_pool(name="kmp", bufs=3) as kmp,
            tc.tile_pool(name="scratch", bufs=2) as scratch,
            tc.tile_pool(name="small", bufs=1) as small,
            tc.tile_pool(name="psum", bufs=4, space=bass.MemorySpace.PSUM) as psp,
            tc.tile_pool(name="dram", bufs=1, space=bass.MemorySpace.DRAM) as dram,
        ):
            xh = singles.tile([128, OHL, 67, CIN], BF16)
            nc.sync.dma_start(out=xh[:], in_=XH.ap().rearrange("p (a b c) -> p a b c", a=OHL, b=67))
            xw = singles.tile([128, OHL, OW, B], BF16)
            nc.sync.dma_start(out=xw[:], in_=XW.ap().rearrange("p (a b c) -> p a b c", a=OHL, b=OW))
            xr = singles.tile([64, OHL, OW, B], BF16)
            nc.sync.dma_start(out=xr[:], in_=XR.ap().rearrange("p (a b c) -> p a b c", a=OHL, b=OW))
            kt = singles.tile([64, NPOS, F], BF16)
            nc.sync.dma_start(out=kt[:], in_=KT.ap().rearrange("p (a b) -> p a b", a=NPOS))
            sc = small.tile([64, 1], F32)
            nc.sync.dma_start(out=sc[:], in_=SC.ap())
            bb = small.tile([64, 1], F32)
            nc.sync.dma_start(out=bb[:], in_=BB.ap())

            y_sb = singles.tile([64, NPOS, B], F32)
            sums = small.tile([64, GROUPS], F32)
            sqsums = small.tile([64, GROUPS], F32)

            kmv = KM.ap().rearrange(
                "p (g q t f) -> p g q t f", g=GROUPS, q=GP, t=6
            )

            for g in range(GROUPS):
                km = kmp.tile([128, GP, 6, F], BF16)
                nc.sync.dma_start(out=km[:], in_=kmv[:, g])
                ps = psp.tile([64, GP, B], F32)
                for pl in range(GP):
                    pos = g * GP + pl
                    ohl, ow = divmod(pos, OW)
                    for t in range(6):
                        rhs = (
                            xh[:, ohl, 2 * ow + t, :]
                            if t < 5
                            else xw[:, ohl, ow, :]
                        )
                        nc.tensor.matmul(
                            ps[:, pl, :],
                            km[:, pl, t, :],
                            rhs,
                            start=(t == 0),
                            stop=False,
                        )
                    nc.tensor.matmul(
                        ps[:, pl, :],
                        kt[:, pos, :],
                        xr[:, ohl, ow, :],
                        start=False,
                        stop=True,
                    )
                # leaky relu drain: y = max(ps, 0.01*ps), accumulate sums
                tmp = scratch.tile([64, GP, B], F32, tag="lr")
                nc.scalar.activation(
                    out=tmp[:], in_=ps[:], func=mybir.ActivationFunctionType.Copy,
                    scale=NEG_SLOPE,
                )
                ysl = y_sb[:, g * GP : (g + 1) * GP, :]
                nc.vector.scalar_tensor_tensor(
                    out=ysl,
                    in0=ps[:],
                    scalar=1.0,
                    in1=tmp[:],
                    op0=mult,
                    op1=amax,
                    accum_out=sums[:, g : g + 1],
                )
                sq = scratch.tile([64, GP, B], F32, tag="sq")
                nc.scalar.activation(
                    out=sq[:], in_=ysl, func=mybir.ActivationFunctionType.Square,
                    accum_out=sqsums[:, g : g + 1],
                )

            # ---- BN stats: local totals -> AllReduce -> normalize ----
            cc_sb = small.tile([64, 2], F32)
            nc.vector.tensor_reduce(
                out=cc_sb[:, 0:1], in_=sums[:], axis=mybir.AxisListType.X, op=aadd
            )
            nc.vector.tensor_reduce(
                out=cc_sb[:, 1:2], in_=sqsums[:], axis=mybir.AxisListType.X, op=aadd
            )
            ccin = dram.tile([64, 2], F32)
            ccout = dram.tile([64, 2], F32)
            nc.sync.dma_start(out=ccin[:], in_=cc_sb[:])
            nc.gpsimd.collective_compute(
                "AllReduce",
                aadd,
                replica_groups=[list(range(NCORES))],
                ins=[ccin.opt()],
                outs=[ccout.opt()],
            )
            tot = small.tile([64, 2], F32)
            nc.sync.dma_start(out=tot[:], in_=ccout[:])

            # mean/meansq
            ms = small.tile([64, 2], F32)
            nc.scalar.activation(
                out=ms[:], in_=tot[:], func=mybir.ActivationFunctionType.Copy,
                scale=1.0 / NTOT,
            )
            var = small.tile([64, 1], F32)
            nc.vector.tensor_mul(var[:], ms[:, 0:1], ms[:, 0:1])
            nc.vector.tensor_sub(var[:], ms[:, 1:2], var[:])
            epst = small.tile([64, 1], F32)
            nc.vector.memset(epst[:], EPS)
            sd = small.tile([64, 1], F32)
            nc.scalar.activation(
                out=sd[:], in_=var[:], func=mybir.ActivationFunctionType.Sqrt,
                bias=epst[:], scale=1.0,
            )
            rstd = small.tile([64, 1], F32)
            nc.vector.reciprocal(out=rstd[:], in_=sd[:])
            av = small.tile([64, 1], F32)
            nc.vector.tensor_mul(av[:], sc[:], rstd[:])
            bv = small.tile([64, 1], F32)
            nc.vector.tensor_mul(bv[:], ms[:, 0:1], av[:])
            nc.vector.tensor_sub(bv[:], bb[:], bv[:])

            yo = singles.tile([64, NPOS, B], F32)
            nc.scalar.activation(
                out=yo[:], in_=y_sb[:],
                func=mybir.ActivationFunctionType.Identity,
                bias=bv[:], scale=av[:],
            )
            nc.sync.dma_start(
                out=Y.ap().rearrange("p (a b) -> p a b", a=NPOS), in_=yo[:]
            )

    nc.compile()
    return nc


_NC_CACHE = None
RUN_KWARGS = {}  # test harness may set e.g. {"trace": True}
LAST_RESULT = None


def kernel(x, kernel, bias, scale, bn_bias):
    global _NC_CACHE, LAST_RESULT
    in_maps = _marshal(x, kernel, bias)
    sc = np.ascontiguousarray(np.asarray(scale, np.float32).reshape(64, 1))
    bb = np.ascontiguousarray(np.asarray(bn_bias, np.float32).reshape(64, 1))
    for m in in_maps:
        m["SC"] = sc
        m["BB"] = bb

    if _NC_CACHE is None:
        _NC_CACHE = _build_nc()
    nc = _NC_CACHE

    res = bass_utils.run_bass_kernel_spmd(
        nc, in_maps, core_ids=list(range(NCORES)), **RUN_KWARGS
    )
    LAST_RESULT = res

    out = np.empty((B, OH, OW, F), np.float32)
    for c in range(NCORES):
        yc = res.results[c]["Y"].reshape(64, OHL, OW, B)  # [f, ohl, ow, b]
        out[:, 4 * c : 4 * c + 4, :, :] = yc.transpose(3, 1, 2, 0)
    return out


# revision 6
# speedup vs baseline: 2.0204x; 1.1437x over previous
"""Trainium2 Bass kernel for nn_LocallyConnectedBlock.

Locally-connected conv (5x5, stride 2, SAME) + bias + leaky_relu(0.01) +
BatchNorm (training mode, batch stats over B,OH,OW).

Sharding: spatial over OH, 4 output rows per core x 8 cores, 128 output
positions per core. Compute orientation: out[b, f] per position, with 4
consecutive positions packed onto the 128 PSUM partitions (4 x 32 batch)
via PE column-group tiling:
  per position, 7 contraction chunks (6x128 + 1x64 incl. bias-as-ones-row):
    matmul(out=psum[32i:32i+32, :], lhsT=patches[K,32], rhs=kernel[K,64])
All matmul inputs bf16 (fp32 PSUM accumulation); leaky relu + BN stats +
normalize on device; [1,128] AllReduce of BN sums across the 8 cores.
Host only marshals layouts (im2col-style shifted replicas of x, kernel
reordering to partition-major, bf16 casts).
"""

import ml_dtypes
import numpy as np

import concourse.bass as bass
import concourse.mybir as mybir
import concourse.tile as tile
from concourse import bacc
from concourse import bass_utils

B, H, W, CIN = 32, 64, 64, 32
KH = KW = 5
F = 64
OH = OW = 32
NCORES = 8
OHL = 4  # output rows per core
NPOS = OHL * OW  # 128 positions per core
NEG_SLOPE = 0.01
EPS = 1e-5
NTOT = float(B * OH * OW)  # BN sample count (32768)
GROUPS = 8
GP = NPOS // GROUPS  # 16 positions per group
QG = GP // 4  # quads per group (4)
NQ = NPOS // 4  # 32 quads per core

F32 = mybir.dt.float32
BF16 = mybir.dt.bfloat16


def _marshal(x, kern, bias):
    """Build the 8 per-core input maps (bf16 for matmul operands)."""
    x = np.ascontiguousarray(x, dtype=np.float32)
    kern = np.ascontiguousarray(kern, dtype=np.float32)
    bias = np.ascontiguousarray(bias, dtype=np.float32)

    # SAME padding for 5x5 stride2: pad_lo=1, pad_hi=2 (verified vs jax)
    xp = np.zeros((B, H + 3, W + 3, CIN), np.float32)
    xp[:, 1 : 1 + H, 1 : 1 + W, :] = x
    # patch(oh,ow,kh,kw,c) = xp[:, 2*oh+kh, 2*ow+kw, c]

    kr = kern.reshape(OH, OW, CIN, KH, KW, F)  # c-major fan_in (verified)

    jj = np.arange(4)
    bf = lambda a: np.ascontiguousarray(a.astype(ml_dtypes.bfloat16))
    in_maps = []
    for c in range(NCORES):
        r0 = 8 * c
        # XH[j*32+ci, ohl, w, b] = xp[b, r0+2*ohl+j, w, ci]   (w in 0..66)
        rows = r0 + 2 * jj[None, :] + jj[:, None]  # [j, ohl]
        t = xp[:, rows, 0:67, :]  # [B, j, ohl, 67, CIN]
        xh = np.ascontiguousarray(t.transpose(1, 4, 2, 3, 0)).reshape(128, -1)

        # rows for kh=4 taps
        rw = r0 + 2 * jj + 4  # [ohl]
        t2 = xp[:, rw, :, :]  # [B, ohl, W+3, CIN]
        # XW[j*32+ci, ohl, ow, b] = xp[b, r0+2*ohl+4, 2*ow+j, ci]
        colidx = 2 * np.arange(OW)[None, :] + jj[:, None]  # [j, ow]
        t3 = t2[:, :, colidx, :]  # [B, ohl, j, ow, CIN]
        xw = np.ascontiguousarray(t3.transpose(2, 4, 1, 3, 0)).reshape(128, -1)

        # XR[ci, ohl, ow, b] = xp[b, r0+2*ohl+4, 2*ow+4, ci]; row32=1; 33..63=0
        t4 = t2[:, :, 2 * np.arange(OW) + 4, :]  # [B, ohl, ow, CIN]
        xr = np.zeros((64, OHL, OW, B), np.float32)
        xr[0:32] = t4.transpose(3, 1, 2, 0)
        xr[32] = 1.0
        xr = xr.reshape(64, -1)

        ks = kr[4 * c : 4 * c + 4]  # [ohl, ow, ci, kh, kw, f]
        # KM[j*32+ci, pos, t, f]: t<5 -> (kh=j, kw=t); t=5 -> (kh=4, kw=j)
        km = np.empty((4, 32, OHL, OW, 6, F), np.float32)  # [j, ci, ohl, ow, t, f]
        for tt in range(5):
            km[:, :, :, :, tt, :] = ks[:, :, :, 0:4, tt, :].transpose(3, 2, 0, 1, 4)
        km[:, :, :, :, 5, :] = ks[:, :, :, 4, 0:4, :].transpose(3, 2, 0, 1, 4)
        km = np.ascontiguousarray(km).reshape(128, NPOS, 6, F).reshape(128, -1)

        # KT[p, pos, f]: p<32 tap(4,4); p=32 bias; rest 0
        kt = np.zeros((64, OHL, OW, F), np.float32)
        kt[0:32] = ks[:, :, :, 4, 4, :].transpose(2, 0, 1, 3)
        kt[32] = bias[4 * c : 4 * c + 4]
        kt = kt.reshape(64, -1)

        in_maps.append(
            {"XH": bf(xh), "XW": bf(xw), "XR": bf(xr), "KM": bf(km), "KT": bf(kt)}
        )
    return in_maps


def _build_nc():
    nc = bacc.Bacc(
        "TRN2",
        target_bir_lowering=False,
        debug=False,
        enable_asserts=False,
        num_devices=NCORES,
    )
    XH = nc.dram_tensor("XH", [128, OHL * 67 * CIN], BF16, kind="ExternalInput")
    XW = nc.dram_tensor("XW", [128, OHL * OW * B], BF16, kind="ExternalInput")
    XR = nc.dram_tensor("XR", [64, OHL * OW * B], BF16, kind="ExternalInput")
    KM = nc.dram_tensor("KM", [128, NPOS * 6 * F], BF16, kind="ExternalInput")
    KT = nc.dram_tensor("KT", [64, NPOS * F], BF16, kind="ExternalInput")
    SC = nc.dram_tensor("SC", [1, F], F32, kind="ExternalInput")
    BB = nc.dram_tensor("BB", [1, F], F32, kind="ExternalInput")
    Y = nc.dram_tensor("Y", [128, NQ * F], F32, kind="ExternalOutput")

    mult = mybir.AluOpType.mult
    amax = mybir.AluOpType.max
    aadd = mybir.AluOpType.add

    with tile.TileContext(nc) as tc:
        with (
            tc.tile_pool(name="singles", bufs=1) as singles,
            tc.tile_pool(name="kmp", bufs=3) as kmp,
            tc.tile_pool(name="scratch", bufs=2) as scratch,
            tc.tile_pool(name="small", bufs=1) as small,
            tc.tile_pool(name="psum", bufs=4, space=bass.MemorySpace.PSUM) as psp,
            tc.tile_pool(name="pse", bufs=1, space=bass.MemorySpace.PSUM) as pse,
            tc.tile_pool(name="dram", bufs=1, space=bass.MemorySpace.DRAM) as dram,
        ):
            xh = singles.tile([128, OHL, 67, CIN], BF16)
            nc.sync.dma_start(
                out=xh[:], in_=XH.ap().rearrange("p (a b c) -> p a b c", a=OHL, b=67)
            )
            xw = singles.tile([128, OHL, OW, B], BF16)
            nc.sync.dma_start(
                out=xw[:], in_=XW.ap().rearrange("p (a b c) -> p a b c", a=OHL, b=OW)
            )
            xr = singles.tile([64, OHL, OW, B], BF16)
            nc.sync.dma_start(
                out=xr[:], in_=XR.ap().rearrange("p (a b c) -> p a b c", a=OHL, b=OW)
            )
            kt = singles.tile([64, NPOS, F], BF16)
            nc.sync.dma_start(
                out=kt[:], in_=KT.ap().rearrange("p (a b) -> p a b", a=NPOS)
            )
            sc = small.tile([1, F], F32)
            nc.sync.dma_start(out=sc[:], in_=SC.ap())
            bb = small.tile([1, F], F32)
            nc.sync.dma_start(out=bb[:], in_=BB.ap())

            y_sb = singles.tile([128, NQ, F], F32)
            fsums = small.tile([128, GROUPS, F], F32)
            fsqs = small.tile([128, GROUPS, F], F32)

            kmv = KM.ap().rearrange("p (g q t f) -> p g q t f", g=GROUPS, q=GP, t=6)

            for g in range(GROUPS):
                km = kmp.tile([128, GP, 6, F], BF16)
                nc.sync.dma_start(out=km[:], in_=kmv[:, g])
                ps = psp.tile([128, QG, F], F32)
                for ql in range(QG):
                    q = g * QG + ql
                    for i in range(4):
                        pos = 4 * q + i
                        pl = pos - g * GP
                        ohl, ow = divmod(pos, OW)
                        out_sl = ps[32 * i : 32 * i + 32, ql, :]
                        tp = (0, 32 * i)
                        for t in range(6):
                            lhsT = (
                                xh[:, ohl, 2 * ow + t, :]
                                if t < 5
                                else xw[:, ohl, ow, :]
                            )
                            nc.tensor.matmul(
                                out_sl,
                                lhsT,
                                km[:, pl, t, :],
                                start=(t == 0),
                                stop=False,
                                tile_position=tp,
                            )
                        nc.tensor.matmul(
                            out_sl,
                            xr[:, ohl, ow, :],
                            kt[:, pos, :],
                            start=False,
                            stop=True,
                            tile_position=tp,
                        )
                # leaky relu drain: y = max(ps, 0.01*ps)
                tmp = scratch.tile([128, QG, F], F32, tag="lr")
                nc.scalar.activation(
                    out=tmp[:],
                    in_=ps[:],
                    func=mybir.ActivationFunctionType.Copy,
                    scale=NEG_SLOPE,
                )
                ysl = y_sb[:, g * QG : (g + 1) * QG, :]
                nc.vector.scalar_tensor_tensor(
                    out=ysl, in0=ps[:], scalar=1.0, in1=tmp[:], op0=mult, op1=amax
                )
                # per-group BN partials: sum over quads (per f)
                nc.vector.tensor_reduce(
                    out=fsums[:, g, :],
                    in_=ysl.rearrange("p q f -> p f q"),
                    axis=mybir.AxisListType.X,
                    op=aadd,
                )
                sq = scratch.tile([128, QG, F], F32, tag="sq")
                nc.scalar.activation(
                    out=sq[:], in_=ysl, func=mybir.ActivationFunctionType.Square
                )
                nc.vector.tensor_reduce(
                    out=fsqs[:, g, :],
                    in_=sq[:].rearrange("p q f -> p f q"),
                    axis=mybir.AxisListType.X,
                    op=aadd,
                )

            # ---- BN stats: per-f totals as [1, 2F] row, AllReduce, A/B ----
            fs = small.tile([128, F], F32)
            nc.vector.tensor_reduce(
                out=fs[:],
                in_=fsums[:].rearrange("p g f -> p f g"),
                axis=mybir.AxisListType.X,
                op=aadd,
            )
            fq = small.tile([128, F], F32)
            nc.vector.tensor_reduce(
                out=fq[:],
                in_=fsqs[:].rearrange("p g f -> p f g"),
                axis=mybir.AxisListType.X,
                op=aadd,
            )
            ones128 = small.tile([128, 1], F32)
            nc.vector.memset(ones128[:], 1.0)
            st_ps = pse.tile([1, 2 * F], F32)
            nc.tensor.matmul(st_ps[:, 0:F], ones128[:], fs[:], start=True, stop=True)
            nc.tensor.matmul(
                st_ps[:, F : 2 * F], ones128[:], fq[:], start=True, stop=True
            )
            cc_sb = small.tile([1, 2 * F], F32)
            nc.scalar.activation(
                out=cc_sb[:], in_=st_ps[:], func=mybir.ActivationFunctionType.Copy
            )
            ccin = dram.tile([1, 2 * F], F32)
            ccout = dram.tile([1, 2 * F], F32)
            nc.sync.dma_start(out=ccin[:], in_=cc_sb[:])
            nc.gpsimd.collective_compute(
                "AllReduce",
                aadd,
                replica_groups=[list(range(NCORES))],
                ins=[ccin.opt()],
                outs=[ccout.opt()],
            )
            tot = small.tile([1, 2 * F], F32)
            nc.sync.dma_start(out=tot[:], in_=ccout[:])

            # mean/meansq rows
            ms = small.tile([1, 2 * F], F32)
            nc.scalar.activation(
                out=ms[:],
                in_=tot[:],
                func=mybir.ActivationFunctionType.Copy,
                scale=1.0 / NTOT,
            )
            var = small.tile([1, F], F32)
            nc.vector.tensor_mul(var[:], ms[:, 0:F], ms[:, 0:F])
            nc.vector.tensor_sub(var[:], ms[:, F : 2 * F], var[:])
            epst = small.tile([1, 1], F32)
            nc.vector.memset(epst[:], EPS)
            sd = small.tile([1, F], F32)
            nc.scalar.activation(
                out=sd[:],
                in_=var[:],
                func=mybir.ActivationFunctionType.Sqrt,
                bias=epst[:],
                scale=1.0,
            )
            ab = small.tile([1, 2 * F], F32)
            nc.vector.reciprocal(out=ab[:, 0:F], in_=sd[:])  # rstd
            nc.vector.tensor_mul(ab[:, 0:F], sc[:], ab[:, 0:F])  # A = scale*rstd
            nc.vector.tensor_mul(ab[:, F : 2 * F], ms[:, 0:F], ab[:, 0:F])  # mean*A
            nc.vector.tensor_sub(ab[:, F : 2 * F], bb[:], ab[:, F : 2 * F])  # B

            # broadcast A|B rows to 128 partitions via K=1 matmul
            one1 = small.tile([1, 128], F32)
            nc.vector.memset(one1[:], 1.0)
            bc_ps = pse.tile([128, 2 * F], F32)
            nc.tensor.matmul(bc_ps[:], one1[:], ab[:], start=True, stop=True)
            absb = small.tile([128, 2 * F], F32)
            nc.scalar.activation(
                out=absb[:], in_=bc_ps[:], func=mybir.ActivationFunctionType.Copy
            )

            # apply: yo = y*A + B  (A,B broadcast over the quad dim)
            a_sl = absb[:, 0:F]
            b_sl = absb[:, F : 2 * F]
            apA = bass.AP(
                tensor=a_sl.tensor,
                offset=a_sl.offset,
                ap=[a_sl.ap[0], [0, NQ], a_sl.ap[1]],
            )
            apB = bass.AP(
                tensor=b_sl.tensor,
                offset=b_sl.offset,
                ap=[b_sl.ap[0], [0, NQ], b_sl.ap[1]],
            )
            yo = singles.tile([128, NQ, F], F32)
            nc.vector.scalar_tensor_tensor(
                out=yo[:], in0=y_sb[:], scalar=1.0, in1=apA, op0=mult, op1=mult
            )
            yo2 = singles.tile([128, NQ, F], F32)
            nc.vector.tensor_add(yo2[:], yo[:], apB)
            nc.sync.dma_start(
                out=Y.ap().rearrange("p (a b) -> p a b", a=NQ), in_=yo2[:]
            )

    nc.compile()
    return nc


_NC_CACHE = None
RUN_KWARGS = {}  # test harness may set e.g. {"trace": True}
LAST_RESULT = None


def kernel(x, kernel, bias, scale, bn_bias):
    global _NC_CACHE, LAST_RESULT
    in_maps = _marshal(x, kernel, bias)
    sc = np.ascontiguousarray(np.asarray(scale, np.float32).reshape(1, F))
    bb = np.ascontiguousarray(np.asarray(bn_bias, np.float32).reshape(1, F))
    for m in in_maps:
        m["SC"] = sc
        m["BB"] = bb

    if _NC_CACHE is None:
        _NC_CACHE = _build_nc()
    nc = _NC_CACHE

    res = bass_utils.run_bass_kernel_spmd(
        nc, in_maps, core_ids=list(range(NCORES)), **RUN_KWARGS
    )
    LAST_RESULT = res

    out = np.empty((B, OH, OW, F), np.float32)
    for c in range(NCORES):
        yc = res.results[c]["Y"].reshape(4, B, NQ, F)  # [i, b, q, f], pos=4q+i
        yb = np.transpose(yc, (1, 2, 0, 3)).reshape(B, OHL, OW, F)
        out[:, 4 * c : 4 * c + 4, :, :] = yb
    return out
